# revision 7
# baseline (speedup 1.0000x reference)
"""Trainium2 kernel for nn_MultiHeadAttention_6511170421145 (sparse attention).

Data-parallel over batch B=16 across the 8 NeuronCores (2 batches/core) via
jax.pmap on the axon-tunneled devices. Weights are broadcast; each core
computes its batch shard of the full heterogeneous-attention forward; results
are gathered to the full (B, G, E) output.
"""

import math

import numpy as np

H, D, DK, E = 8, 256, 32, 256
B, G = 16, 513
NP = (G - 1) // 2
NCORES = 8
NORM = 1.0 / math.sqrt(DK)

_PMAP = None


def _build_pmap():
    import jax
    import jax.numpy as jnp

    def _proj(x, W):
        # x: (b, N, D), W: (H, D, K) -> (H, b, N, K)
        return jnp.einsum("bnd,hdk->hbnk", x, W)

    def fwd(q, h, W_query, W_key, W_val, W1_query, W2_query, W3_query,
            W4_query, W5_query, W6_query, W_out):
        dt = q.dtype
        ninf = lambda *s: jnp.full(s, -jnp.inf, dt)
        zeros = lambda *s: jnp.zeros(s, dt)
        b = q.shape[0]

        Q = _proj(q, W_query)
        K = _proj(h, W_key)
        V = _proj(h, W_val)

        pick = h[:, 1 : NP + 1, :]
        delivery = h[:, NP + 1 :, :]

        Kp = K[:, :, 1 : NP + 1, :]
        Kd = K[:, :, NP + 1 :, :]
        Vp = V[:, :, 1 : NP + 1, :]
        Vd = V[:, :, NP + 1 :, :]

        Q1 = _proj(pick, W1_query)
        Q2 = _proj(pick, W2_query)
        Q3 = _proj(pick, W3_query)
        Q4 = _proj(delivery, W4_query)
        Q5 = _proj(delivery, W5_query)
        Q6 = _proj(delivery, W6_query)

        compat = NORM * jnp.einsum("hbqk,hbnk->hbqn", Q, K)
        c_pd = NORM * jnp.sum(Q1 * Kd, axis=-1)
        c_dp = NORM * jnp.sum(Q4 * Kp, axis=-1)
        c_p_ap = NORM * jnp.einsum("hbqk,hbnk->hbqn", Q2, Kp)
        c_p_ad = NORM * jnp.einsum("hbqk,hbnk->hbqn", Q3, Kd)
        c_d_ad = NORM * jnp.einsum("hbqk,hbnk->hbqn", Q5, Kd)
        c_d_ap = NORM * jnp.einsum("hbqk,hbnk->hbqn", Q6, Kp)

        add_delivery = jnp.concatenate(
            [ninf(H, b, 1), c_pd, ninf(H, b, NP)], axis=-1)[..., None]
        add_allpick = jnp.concatenate(
            [ninf(H, b, 1, NP), c_p_ap, ninf(H, b, NP, NP)], axis=2)
        add_alldelivery = jnp.concatenate(
            [ninf(H, b, 1, NP), c_p_ad, ninf(H, b, NP, NP)], axis=2)
        add_pick = jnp.concatenate(
            [ninf(H, b, 1 + NP), c_dp], axis=-1)[..., None]
        add_alldelivery2 = jnp.concatenate(
            [ninf(H, b, 1 + NP, NP), c_d_ad], axis=2)
        add_allpick2 = jnp.concatenate(
            [ninf(H, b, 1 + NP, NP), c_d_ap], axis=2)

        compat_full = jnp.concatenate(
            [compat, add_delivery, add_allpick, add_alldelivery,
             add_pick, add_alldelivery2, add_allpick2], axis=-1)
        attn = jax.nn.softmax(compat_full, axis=-1)

        s0 = G
        V_add_d = jnp.concatenate(
            [zeros(H, b, 1, DK), Vd, zeros(H, b, NP, DK)], axis=2)
        V_add_p = jnp.concatenate([zeros(H, b, 1 + NP, DK), Vp], axis=2)

        heads = jnp.einsum("hbqn,hbnv->hbqv", attn[..., :s0], V)
        heads = heads + attn[..., s0 : s0 + 1] * V_add_d
        heads = heads + jnp.einsum(
            "hbqn,hbnv->hbqv", attn[..., s0 + 1 : s0 + 1 + NP], Vp)
        heads = heads + jnp.einsum(
            "hbqn,hbnv->hbqv", attn[..., s0 + 1 + NP : s0 + 1 + 2 * NP], Vd)
        heads = heads + attn[..., s0 + 1 + 2 * NP : s0 + 2 + 2 * NP] * V_add_p
        heads = heads + jnp.einsum(
            "hbqn,hbnv->hbqv", attn[..., s0 + 2 + 2 * NP : s0 + 2 + 3 * NP], Vd)
        heads = heads + jnp.einsum(
            "hbqn,hbnv->hbqv", attn[..., s0 + 2 + 3 * NP :], Vp)

        return jnp.einsum("hbnv,hve->bne", heads, W_out)

    wnames = ("W_query", "W_key", "W_val", "W1_query", "W2_query", "W3_query",
              "W4_query", "W5_query", "W6_query", "W_out")
    f = jax.pmap(
        fwd,
        axis_name="c",
        in_axes=(0, 0) + (None,) * len(wnames),
        devices=jax.devices()[:NCORES],
    )
    return f


def kernel(**inputs: np.ndarray) -> np.ndarray:
    global _PMAP
    if _PMAP is None:
        _PMAP = _build_pmap()

    q = np.asarray(inputs["q"], dtype=np.float32).reshape(NCORES, B // NCORES, G, D)
    h = np.asarray(inputs["h"], dtype=np.float32).reshape(NCORES, B // NCORES, G, D)
    ws = [np.asarray(inputs[k], dtype=np.float32) for k in
          ("W_query", "W_key", "W_val", "W1_query", "W2_query", "W3_query",
           "W4_query", "W5_query", "W6_query", "W_out")]
    out = _PMAP(q, h, *ws)
    return np.asarray(out).reshape(B, G, E)


if __name__ == "__main__":
    rng = np.random.default_rng(0)
    ins = {
        "q": rng.standard_normal((B, G, D), dtype=np.float32),
        "h": rng.standard_normal((B, G, D), dtype=np.float32),
        **{n: rng.uniform(-0.17, 0.17, (H, D, DK)).astype(np.float32)
           for n in ["W_query", "W_key", "W_val", "W1_query", "W2_query",
                     "W3_query", "W4_query", "W5_query", "W6_query"]},
        "W_out": rng.uniform(-0.06, 0.06, (H, DK, E)).astype(np.float32),
    }
    o = kernel(**ins)
    print(o.shape, o.dtype, float(np.abs(o).mean()))
